# revision 3
# baseline (speedup 1.0000x reference)
"""Decode-step multi-head attention (KV cache) on 8 Trainium2 NeuronCores.

Full inputs in, full outputs out.  Tensor-parallel over heads: each of the
8 cores owns 4 of the 32 heads (wq/wk/wv column shards, wo row shard, head
slice of both KV caches).  The all-reduce after the output projection and
the head-axis gather of the attention weights happen on the host.

Problem shape (hardcoded):
  x[16,1,4096], wq/wk/wv/wo[4096,4096], cache_k/v[16,4096,32,128],
  mask[1,1,1,4096], start_pos=4095, H=32 heads, dqk=dv=128.
Returns (weight[16,32,1,4096], output[16,1,4096]) like the reference.
"""

import numpy as np

BS = 16
DIM = 4096
H = 32
DQK = 128
DV = 128
MAX_SEQ = 4096
START_POS = 4095
NCORES = 8
HL = H // NCORES          # 4 local heads per core
NPAIR = HL * BS           # 64 (head, batch) pairs per core; pair = h*16 + b
NTILE = MAX_SEQ // 128    # 32 seq tiles of 128
SCALE = DQK ** -0.5

_CACHE = {}


def _build_nc():
    """Build the per-core Bass program (same program on all 8 cores)."""
    from contextlib import ExitStack

    import concourse.mybir as mybir
    import concourse.tile as tile
    from concourse import bacc

    f32 = mybir.dt.float32
    # Bacc (not plain Bass): finalize() -> compile() runs the wait-splitting
    # passes (move_matmul_waits_to_ldweights / generate_event_semaphores)
    # without which walrus rejects multi-wait matmuls ("Too many sync wait
    # commands" in setupSyncWait<S3_LW>).
    nc = bacc.Bacc()

    # ---- DRAM parameters (per-core shards, host-prepared layouts) ----
    # xT[p, k*16+b] = x[b, k*128+p]
    xT_d = nc.declare_dram_parameter("xT", [128, NTILE * BS], f32, isOutput=False)
    # maskT[p, j] = mask[j*128+p]
    maskT_d = nc.declare_dram_parameter("maskT", [128, NTILE], f32, isOutput=False)
    wq_d = nc.declare_dram_parameter("wq", [DIM, HL * DQK], f32, isOutput=False)
    wk_d = nc.declare_dram_parameter("wk", [DIM, HL * DQK], f32, isOutput=False)
    wv_d = nc.declare_dram_parameter("wv", [DIM, HL * DV], f32, isOutput=False)
    wo_d = nc.declare_dram_parameter("wo", [HL * DV, DIM], f32, isOutput=False)
    # kT[pair, d, s] = cache_k[b, s, head, d]   (pair = h_local*16 + b)
    kT_d = nc.declare_dram_parameter("kT", [NPAIR, DQK, MAX_SEQ], f32, isOutput=False)
    # v[pair, s, d] = cache_v[b, s, head, d]
    v_d = nc.declare_dram_parameter("v", [NPAIR, MAX_SEQ, DV], f32, isOutput=False)
    # weight_out[pair, p, j] = scores(s = j*128+p) + mask
    weight_d = nc.declare_dram_parameter(
        "weight", [NPAIR, 128, NTILE], f32, isOutput=True
    )
    out_d = nc.declare_dram_parameter("out", [BS, DIM], f32, isOutput=True)

    Exp = mybir.ActivationFunctionType.Exp

    with tile.TileContext(nc) as tc, ExitStack() as ctx:
        singles = ctx.enter_context(tc.tile_pool(name="singles", bufs=1))
        wpool = ctx.enter_context(tc.tile_pool(name="wpool", bufs=3))
        kpool = ctx.enter_context(tc.tile_pool(name="kpool", bufs=3))
        vpool = ctx.enter_context(tc.tile_pool(name="vpool", bufs=3))
        spool = ctx.enter_context(tc.tile_pool(name="spool", bufs=3))
        ppool = ctx.enter_context(tc.tile_pool(name="ppool", bufs=2, space="PSUM"))
        scorep = ctx.enter_context(tc.tile_pool(name="scorep", bufs=3, space="PSUM"))
        ctxp = ctx.enter_context(tc.tile_pool(name="ctxp", bufs=1, space="PSUM"))
        redp = ctx.enter_context(tc.tile_pool(name="redp", bufs=2, space="PSUM"))

        # ---- persistent SBUF ----
        xT_sb = singles.tile([128, NTILE * BS], f32, tag="xT")
        nc.sync.dma_start(out=xT_sb, in_=xT_d[:])
        maskT_sb = singles.tile([128, NTILE], f32, tag="maskT")
        nc.sync.dma_start(out=maskT_sb, in_=maskT_d[:])

        qT_sb = singles.tile([128, NPAIR], f32, tag="qT")       # q^T, pre-scaled
        kTn_sb = singles.tile([128, NPAIR], f32, tag="kTn")     # new key (s=4095)^T
        xv_sb = singles.tile([BS, HL * DV], f32, tag="xv")      # new value rows
        e_sb = singles.tile([128, NPAIR * NTILE], f32, tag="e") # exp(scores)
        rowsum_sb = singles.tile([128, NPAIR], f32, tag="rowsum")
        ones_col = singles.tile([128, 1], f32, tag="ones_col")
        nc.vector.memset(ones_col, 1.0)
        ones_row = singles.tile([1, 128], f32, tag="ones_row")
        nc.vector.memset(ones_row, 1.0)
        recip_sb = singles.tile([1, NPAIR], f32, tag="recip")
        recipb_sb = singles.tile([128, NPAIR], f32, tag="recipb")
        ctx_sb = singles.tile([128, NPAIR], f32, tag="ctxsb")
        out_sb = singles.tile([BS, DIM], f32, tag="outsb")

        # ---- QKV projections (weights stationary -> transposed outputs) ----
        # q^T and k_new^T per head block: psum[d(128), b(16)]
        for wdram, dest, scl in ((wq_d, qT_sb, SCALE), (wk_d, kTn_sb, 1.0)):
            for h in range(HL):
                wh = wpool.tile([128, NTILE, 128], f32, tag="wpool")
                nc.sync.dma_start(
                    out=wh,
                    in_=wdram[:, h * 128 : (h + 1) * 128].rearrange(
                        "(k p) c -> p k c", p=128
                    ),
                )
                pq = ppool.tile([128, BS], f32, tag="ppool")
                for k in range(NTILE):
                    nc.tensor.matmul(
                        pq,
                        lhsT=wh[:, k, :],
                        rhs=xT_sb[:, k * BS : (k + 1) * BS],
                        start=(k == 0),
                        stop=(k == NTILE - 1),
                    )
                if scl != 1.0:
                    nc.scalar.mul(dest[:, h * BS : (h + 1) * BS], pq, scl)
                else:
                    nc.vector.tensor_copy(dest[:, h * BS : (h + 1) * BS], pq)

        # v_new in row layout [b, h*128+d] (moving weights)
        pv = ppool.tile([BS, 512], f32, tag="ppool")
        for kk in range(4):
            wv_t = wpool.tile([128, 8, 512], f32, tag="wpool")
            nc.sync.dma_start(
                out=wv_t,
                in_=wv_d[kk * 1024 : (kk + 1) * 1024, :].rearrange(
                    "(k p) c -> p k c", p=128
                ),
            )
            for k2 in range(8):
                kt = kk * 8 + k2
                nc.tensor.matmul(
                    pv,
                    lhsT=xT_sb[:, kt * BS : (kt + 1) * BS],
                    rhs=wv_t[:, k2, :],
                    start=(kt == 0),
                    stop=(kt == NTILE - 1),
                )
        nc.scalar.copy(xv_sb, pv)

        # ---- scores sweep: one K slab (2MB) per pair ----
        for pair in range(NPAIR):
            ks = kpool.tile([128, MAX_SEQ], f32, tag="kpool")
            nc.sync.dma_start(out=ks, in_=kT_d[pair])
            # cache append: overwrite column s=4095 with the new key
            nc.vector.tensor_copy(ks[:, 4095:4096], kTn_sb[:, pair : pair + 1])
            ps = scorep.tile([128, NTILE], f32, tag="scorep")
            for j in range(NTILE):
                nc.tensor.matmul(
                    ps[:, j : j + 1],
                    lhsT=ks[:, j * 128 : (j + 1) * 128],
                    rhs=qT_sb[:, pair : pair + 1],
                    start=True,
                    stop=True,
                )
            ssb = spool.tile([128, NTILE], f32, tag="spool")
            nc.vector.tensor_add(ssb, ps, maskT_sb)
            nc.sync.dma_start(out=weight_d[pair], in_=ssb)
            nc.scalar.activation(
                e_sb[:, pair * NTILE : (pair + 1) * NTILE],
                ssb,
                Exp,
                accum_out=rowsum_sb[:, pair : pair + 1],
            )

        # ---- softmax denominators: partition-sum via ones matmul ----
        pd = redp.tile([1, NPAIR], f32, tag="redp")
        nc.tensor.matmul(pd, lhsT=ones_col, rhs=rowsum_sb, start=True, stop=True)
        nc.vector.reciprocal(recip_sb, pd)
        pb = redp.tile([128, NPAIR], f32, tag="redp")
        nc.tensor.matmul(pb, lhsT=ones_row, rhs=recip_sb, start=True, stop=True)
        nc.vector.tensor_copy(recipb_sb, pb)

        # ---- ctx sweep: one V slab (2MB) per pair, V stationary ----
        pc = ctxp.tile([128, NPAIR], f32, tag="ctxp")
        for pair in range(NPAIR):
            h, b = divmod(pair, BS)
            vs = vpool.tile([128, NTILE, 128], f32, tag="vpool")
            nc.sync.dma_start(out=vs, in_=v_d[pair].rearrange("(j q) d -> q j d", q=128))
            # cache append: overwrite row s=4095 with the new value
            nc.sync.dma_start(
                out=vs[127:128, NTILE - 1, :],
                in_=xv_sb[b : b + 1, h * 128 : (h + 1) * 128],
            )
            for j in range(NTILE):
                nc.tensor.matmul(
                    pc[:, pair : pair + 1],
                    lhsT=vs[:, j, :],
                    rhs=e_sb[:, pair * NTILE + j : pair * NTILE + j + 1],
                    start=(j == 0),
                    stop=(j == NTILE - 1),
                )
        nc.vector.tensor_mul(ctx_sb, pc, recipb_sb)

        # ---- output projection: out[b, dim] = sum_h ctx[:, h] @ wo rows ----
        for c in range(8):
            po = ppool.tile([BS, 512], f32, tag="ppool")
            for h in range(HL):
                wo_t = wpool.tile([128, 512], f32, tag="wpool")
                nc.sync.dma_start(
                    out=wo_t, in_=wo_d[h * 128 : (h + 1) * 128, c * 512 : (c + 1) * 512]
                )
                nc.tensor.matmul(
                    po,
                    lhsT=ctx_sb[:, h * BS : (h + 1) * BS],
                    rhs=wo_t,
                    start=(h == 0),
                    stop=(h == HL - 1),
                )
            nc.scalar.copy(out_sb[:, c * 512 : (c + 1) * 512], po)
        nc.sync.dma_start(out=out_d[:], in_=out_sb)

    nc.finalize()
    return nc


def _get_nc():
    if "nc" not in _CACHE:
        _CACHE["nc"] = _build_nc()
    return _CACHE["nc"]


def _make_in_maps(inputs):
    x = np.ascontiguousarray(np.asarray(inputs["x"], dtype=np.float32))
    mask = np.ascontiguousarray(np.asarray(inputs["mask"], dtype=np.float32))
    wq = np.asarray(inputs["wq"], dtype=np.float32)
    wk = np.asarray(inputs["wk"], dtype=np.float32)
    wv = np.asarray(inputs["wv"], dtype=np.float32)
    wo = np.asarray(inputs["wo"], dtype=np.float32)
    cache_k = np.asarray(inputs["cache_k"], dtype=np.float32)
    cache_v = np.asarray(inputs["cache_v"], dtype=np.float32)

    # x[b, 0, dim] -> xT[p, k*16+b] with dim = k*128+p
    xT = np.ascontiguousarray(
        x.reshape(BS, NTILE, 128).transpose(2, 1, 0).reshape(128, NTILE * BS)
    )
    # mask[..., s] -> maskT[p, j] with s = j*128+p
    maskT = np.ascontiguousarray(mask.reshape(NTILE, 128).T)

    in_maps = []
    for c in range(NCORES):
        cols = slice(c * HL * DQK, (c + 1) * HL * DQK)
        ck = cache_k[:, :, c * HL : (c + 1) * HL, :]  # [16, 4096, 4, 128]
        cv = cache_v[:, :, c * HL : (c + 1) * HL, :]
        kT_c = np.ascontiguousarray(ck.transpose(2, 0, 3, 1)).reshape(
            NPAIR, DQK, MAX_SEQ
        )
        v_c = np.ascontiguousarray(cv.transpose(2, 0, 1, 3)).reshape(
            NPAIR, MAX_SEQ, DV
        )
        in_maps.append(
            {
                "xT": xT,
                "maskT": maskT,
                "wq": np.ascontiguousarray(wq[:, cols]),
                "wk": np.ascontiguousarray(wk[:, cols]),
                "wv": np.ascontiguousarray(wv[:, cols]),
                "wo": np.ascontiguousarray(wo[cols, :]),
                "kT": kT_c,
                "v": v_c,
            }
        )
    return in_maps


def _run(inputs, trace=False, trace_cores=None):
    from concourse.bass_utils import run_bass_kernel_spmd

    nc = _get_nc()
    in_maps = _make_in_maps(inputs)
    res = run_bass_kernel_spmd(
        nc,
        in_maps,
        list(range(NCORES)),
        trace=trace,
        trace_cores=trace_cores,
    )

    weights = []
    out = np.zeros((BS, DIM), dtype=np.float32)
    for c in range(NCORES):
        r = res.results[c]
        w = np.asarray(r["weight"])  # [64, 128, 32]
        w = (
            w.reshape(HL, BS, 128, NTILE)
            .transpose(1, 0, 3, 2)
            .reshape(BS, HL, MAX_SEQ)
        )
        weights.append(w)
        out += np.asarray(r["out"])
    weight = np.concatenate(weights, axis=1)[:, :, None, :]  # [16, 32, 1, 4096]
    output = out[:, None, :]  # [16, 1, 4096]
    return (weight.astype(np.float32), output.astype(np.float32)), res


def kernel(**inputs):
    (weight, output), _ = _run(inputs, trace=False)
    return weight, output
